# revision 8
# baseline (speedup 1.0000x reference)
"""Trainium2 Bass kernel for nn_ListwiseSmoothINDCGKLoss.

Full inputs: s (16384, 2048) f32, label (16384, 2048) f32 (integer values 0..4).
Output: scalar f32 loss = sum_i (1 - ndcg_i).

Strategy (data-parallel over batch rows, 8 cores x 2048 rows):
  Per 128-row tile, keep everything resident in SBUF/PSUM and run the
  K=10 smooth-softmax scan with fused ops:
    - ACT: e = Exp(+/-P - m) with free-axis accum -> Sum(e) in the same pass
      (e written to PSUM so DVE reads it via the PSUM port, leaving the
      shared SBUF port free for GPSIMD).
    - DVE tensor_tensor_reduce: le = label*e with accum -> Sum(label*e).
    - P-update P <- (e*r - 0.9)*P via the fused affine_mul_reduce custom
      DVE op on columns [0, XSPLIT), and via ACT-Copy (q = e*r - 0.9) +
      GPSIMD tensor_tensor (P*q) on columns [XSPLIT, 2048) so all three
      elementwise engines run concurrently.
      The sign trick: we track P_tilde with P_tilde_{k+1} = (e*r - 0.9)*P_tilde_k
      = -P_true_{k+1}, and alternate the exp input scale +/-1 per iteration.
    - iDCG via exact label counts: N_j = #(label >= j), j=1..4, and
      idcg = D(10) + sum_j 2^(j-1) * D(min(N_j, 10)) + EPS with
      D(n) = sum_{k<n} 1/log2(k+2)  (exact for integer labels in 0..4).
  Per-core output: [128,1] per-partition sums of ndcg; host computes
  16384 - sum(all) (exact rewrite of sum(1 - ndcg)).
"""

import math
from contextlib import ExitStack

import numpy as np

ALPHA = 10.0
DELTA = 0.1
K = 10
EPS = 1e-10
LN2 = 0.6931471805599453

BS, LL = 16384, 2048
NCORES = 8
ROWS = BS // NCORES          # rows per core
P = 128                      # partitions
NT = ROWS // P               # row-tiles per core

# Column split for the P-update: [0, XSPLIT) handled by the fused DVE
# affine_mul_reduce; [XSPLIT, LL) by ACT(q=e*r-0.9) + GPSIMD(P*q).
XSPLIT = 512
# Engine for the label>=j counts: "dve" (tensor_scalar is_ge + accum),
# "pool" (same op on GPSIMD) or "act" (Sign trick).
COUNTS_ENGINE = "dve"

_CACHE = {}


def _d_table():
    w = 1.0 / np.log2(np.arange(2.0, K + 2.0, dtype=np.float64))
    D = np.concatenate([[0.0], np.cumsum(w)])
    return w.astype(np.float32), D.astype(np.float32)


def _build_nc():
    import concourse.bass as bass
    import concourse.bacc as bacc
    import concourse.mybir as mybir
    import concourse.tile as tile

    f32 = mybir.dt.float32
    Alu = mybir.AluOpType
    Act = mybir.ActivationFunctionType
    AX = mybir.AxisListType

    w10, D = _d_table()
    D10 = float(D[10])

    nc = bacc.Bacc("TRN2", target_bir_lowering=False, debug=False)

    s_d = nc.dram_tensor("s", [ROWS, LL], f32, kind="ExternalInput")
    lab_d = nc.dram_tensor("label", [ROWS, LL], f32, kind="ExternalInput")
    out_d = nc.dram_tensor("out", [P, 1], f32, kind="ExternalOutput")

    # Baked constants.
    # wI[p, (j-1)*10 + k] = 2^(j-1) / log2(k+2)  -- iDCG weights
    wI_np = np.concatenate([(2.0 ** (j - 1)) * w10 for j in range(1, 5)])
    wI_c = nc.inline_tensor(np.broadcast_to(wI_np, (P, 40)).copy(), name="wI")
    w10_c = nc.inline_tensor(np.broadcast_to(w10, (P, 10)).copy(), name="w10")
    iota_c = nc.inline_tensor(
        np.broadcast_to(np.arange(10, dtype=np.float32), (P, 10)).copy(), name="iota10"
    )

    sap = s_d.ap()
    lap = lab_d.ap()

    with tile.TileContext(nc) as tc, ExitStack() as ctx:
        singles = ctx.enter_context(tc.tile_pool(name="singles", bufs=1))
        io = ctx.enter_context(tc.tile_pool(name="io", bufs=2))
        work = ctx.enter_context(tc.tile_pool(name="work", bufs=1))
        scr = ctx.enter_context(tc.tile_pool(name="scr", bufs=2))
        st = ctx.enter_context(tc.tile_pool(name="st", bufs=2))
        pp = ctx.enter_context(tc.tile_pool(name="pp", bufs=2, space="PSUM"))

        wI_sb = singles.tile([P, 40], f32)
        nc.sync.dma_start(out=wI_sb, in_=wI_c.ap())
        w10_sb = singles.tile([P, 10], f32)
        nc.sync.dma_start(out=w10_sb, in_=w10_c.ap())
        iota_sb = singles.tile([P, 10], f32)
        nc.sync.dma_start(out=iota_sb, in_=iota_c.ap())

        acc = singles.tile([P, 1], f32)
        nc.vector.memset(acc, 0.0)

        for t in range(NT):
            r0 = t * P
            s_sb = io.tile([P, LL], f32, tag="s_sb")
            nc.sync.dma_start(out=s_sb, in_=sap[r0 : r0 + P, :])
            lab_sb = io.tile([P, LL], f32, tag="lab_sb")
            nc.sync.dma_start(out=lab_sb, in_=lap[r0 : r0 + P, :])

            # --- per-tile prep -------------------------------------------
            rmax = st.tile([P, 1], f32, tag="rmax")
            nc.vector.tensor_reduce(rmax, s_sb, axis=AX.X, op=Alu.max)
            rmin = st.tile([P, 1], f32, tag="rmin")
            nc.vector.tensor_reduce(rmin, s_sb, axis=AX.X, op=Alu.min)
            bias1 = st.tile([P, 1], f32, tag="bias1")       # -ALPHA*rmax
            nc.vector.tensor_scalar_mul(bias1, rmax, -ALPHA)
            nrm10 = st.tile([P, 1], f32, tag="nrm10")       # -ALPHA*rmin
            nc.vector.tensor_scalar_mul(nrm10, rmin, -ALPHA)
            nbm = st.tile([P, 1], f32, tag="nbm")           # -m = 10*rmin - 10*rmax
            nc.vector.tensor_sub(nbm, bias1, nrm10)

            Pa = work.tile([P, LL], f32, tag="Pa")
            Pb = work.tile([P, LL], f32, tag="Pb")
            # P1 = ALPHA*s - ALPHA*rmin
            nc.vector.tensor_scalar(Pa, s_sb, ALPHA, nrm10, Alu.mult, Alu.add)

            # --- counts for iDCG -----------------------------------------
            mask40 = st.tile([P, 40], f32, tag="mask40")
            for j in range(1, 5):
                nj = st.tile([P, 1], f32, tag=f"nj{j}")
                csc = scr.tile([P, LL], f32, tag="csc")
                eng = nc.vector if COUNTS_ENGINE == "dve" else nc.gpsimd
                eng.tensor_scalar(
                    csc, lab_sb, float(j), None, Alu.is_ge, Alu.add, accum_out=nj
                )
                # mask40[:, (j-1)*10 + k] = (k < N_j)
                nc.vector.tensor_scalar(
                    mask40[:, (j - 1) * 10 : j * 10], iota_sb, nj, None, Alu.is_lt
                )
            idcg = st.tile([P, 1], f32, tag="idcg")
            m40s = st.tile([P, 40], f32, tag="m40s")
            nc.vector.scalar_tensor_tensor(
                out=m40s, in0=mask40, scalar=1.0, in1=wI_sb,
                op0=Alu.mult, op1=Alu.mult, accum_out=idcg,
            )
            nc.vector.tensor_scalar_add(idcg, idcg, float(D10 + EPS))
            iidcg = st.tile([P, 1], f32, tag="iidcg")
            nc.vector.reciprocal(iidcg, idcg)

            # --- K-step scan ---------------------------------------------
            rel = st.tile([P, 16], f32, tag="rel")
            sgn = 1.0
            for k in range(K):
                e = pp.tile([P, LL], f32, tag="e")
                se = st.tile([P, 1], f32, tag="se")
                if k == 0:
                    nc.scalar.activation(
                        e, s_sb, Act.Exp, bias=bias1, scale=ALPHA, accum_out=se
                    )
                else:
                    nc.scalar.activation(
                        e, Pa, Act.Exp, bias=nbm, scale=sgn, accum_out=se
                    )
                le = scr.tile([P, LL], f32, tag="le")
                sle = st.tile([P, 1], f32, tag="sle")
                nc.vector.scalar_tensor_tensor(
                    out=le, in0=lab_sb, scalar=1.0, in1=e,
                    op0=Alu.mult, op1=Alu.mult, accum_out=sle,
                )
                r = st.tile([P, 1], f32, tag="r")
                nc.vector.reciprocal(r, se)
                nc.vector.tensor_mul(rel[:, k : k + 1], sle, r)
                if k < K - 1:
                    dummy = st.tile([P, 1], f32, tag="dummy")
                    if XSPLIT > 0:
                        nc.vector.affine_mul_reduce(
                            out=Pb[:, :XSPLIT], accum_out=dummy,
                            in0=e[:, :XSPLIT], in1=Pa[:, :XSPLIT],
                            scale=r, bias=-(1.0 - DELTA),
                        )
                    if XSPLIT < LL:
                        q = scr.tile([P, LL - XSPLIT], f32, tag="q")
                        nc.scalar.activation(
                            q, e[:, XSPLIT:], Act.Copy, bias=-(1.0 - DELTA), scale=r
                        )
                        nc.gpsimd.tensor_tensor(
                            Pb[:, XSPLIT:], Pa[:, XSPLIT:], q, Alu.mult
                        )
                    Pa, Pb = Pb, Pa
                    sgn = -sgn

            # --- DCG + loss ----------------------------------------------
            e2 = st.tile([P, 10], f32, tag="e2")
            nc.scalar.activation(e2, rel[:, 0:10], Act.Exp, bias=0.0, scale=LN2)
            d10s = st.tile([P, 10], f32, tag="d10s")
            dcg = st.tile([P, 1], f32, tag="dcg")
            nc.vector.scalar_tensor_tensor(
                out=d10s, in0=e2, scalar=1.0, in1=w10_sb,
                op0=Alu.mult, op1=Alu.mult, accum_out=dcg,
            )
            nc.vector.tensor_scalar_add(dcg, dcg, float(EPS))
            ndcg = st.tile([P, 1], f32, tag="ndcg")
            nc.vector.tensor_mul(ndcg, dcg, iidcg)
            nc.vector.tensor_add(acc, acc, ndcg)

        nc.sync.dma_start(out=out_d.ap(), in_=acc)

    nc.finalize()
    return nc


def _get_nc():
    if "nc" not in _CACHE:
        _CACHE["nc"] = _build_nc()
    return _CACHE["nc"]


def run_cores(s, label):
    """Run the SPMD kernel; returns list of per-core [128,1] ndcg partial sums."""
    from concourse.bass_utils import run_bass_kernel_spmd

    nc = _get_nc()
    s = np.ascontiguousarray(s, dtype=np.float32)
    label = np.ascontiguousarray(label, dtype=np.float32)
    in_maps = [
        {
            "s": s[c * ROWS : (c + 1) * ROWS],
            "label": label[c * ROWS : (c + 1) * ROWS],
        }
        for c in range(NCORES)
    ]
    res = run_bass_kernel_spmd(nc, in_maps, core_ids=list(range(NCORES)))
    return [res.results[c]["out"] for c in range(NCORES)]


def kernel(s, label):
    outs = run_cores(s, label)
    total = np.concatenate([o.reshape(-1) for o in outs]).astype(np.float64).sum()
    return np.float32(float(BS) - total)
